# revision 7
# baseline (speedup 1.0000x reference)
"""AddRandomWalkEdge on 8 TRN2 NeuronCores.

Strategy (per the sharding hint): shard the 400k walkers across the 8 cores,
replicate the CSR-derived table on each core, concatenate per-core walk
outputs on the host.

Host side (graph preprocessing only — no sampling):
  - stable-sort edges by source to build CSR (matches jnp.argsort(stable)),
  - build a fused per-edge table T[e] = (col_s[e] | deg[col_s[e]]<<17,
    rowptr[col_s[e]]) packed in 8 bytes, so each walk step needs exactly ONE
    8-byte gather per walker,
  - isolated nodes get sentinel table rows so the device loop is branch-free.

Device side (the sampling): for each of 8 steps, VectorE computes the edge
index e = rowptr[cur] + min(floor(u * deg), max(deg-1, 0)) for each walker,
then indirect DMAs gather T[e] from HBM. The floor is computed with a
rounding-mode-agnostic correction (cast, compare, subtract) so it matches
numpy/XLA truncation exactly.

The walrus dynamic-DMA contract (established by probing this compiler):
one instruction consumes ONE offset per dest partition (the first element
of that partition's offset-AP row) and copies a contiguous elems-sized run
from table[offset] into that partition's dest run. So a 50176-walker gather
needs 392 instructions of 128 walkers each (dest g[:, 2i:2i+2], offsets
e[:, i:i+1]). More than ~320 dynamic DMAs in flight crash the device, so an
all-engine barrier is inserted every 128 gathers.
"""

import os

import numpy as np

import concourse.bacc as bacc
import concourse.bass as bass
import concourse.mybir as mybir
import concourse.tile as tile
from concourse.bass_utils import run_bass_kernel_spmd

N = 100000
E = 3_200_000
WALKS_PER_NODE = 4
L_TOTAL = 8
S = N * WALKS_PER_NODE
NCORES = 8
SPC = S // NCORES  # walkers per core
P = 128
C = 392  # walker columns per plane; 128*392 = 50176 >= 50000
SPAD = P * C
SENT_PAD = 128  # sentinel rows for isolated nodes
DEG_SHIFT = 17  # word0 = col | deg << 17 (col < 2^17, deg < 2^7)
BARRIER_EVERY = 128

I32 = mybir.dt.int32
F32 = mybir.dt.float32

LAST_EXEC_TIME_NS = None
LAST_RESULTS = None


def _indirect_gather_flat(gp, out_ap, in_ap, offset_ap):
    """Dynamic DMA: per dest partition p, copy `elems` contiguous elements
    from in_[off_p] (off_p = first element of partition p's offset-AP row)
    into that partition's dest run. Bass's stock indirect_dma_start with the
    dest-space assert relaxed."""
    out_l = gp.lower_ap_dma(out_ap, for_indirect_dma=True)
    in_l = gp.lower_ap_dma(in_ap, for_indirect_dma=True)
    assert len(in_l) == 1 and len(out_l) == 1
    off_l = gp.lower_ap_dma(offset_ap)
    assert len(off_l) == 1

    coef = 1
    for i in range(1, len(in_ap.shape)):
        coef *= in_ap.shape[i]

    in_l[0].dynamic_ap_info = mybir.DynamicAccessPatternInfo(
        c=0,
        actual_ap=out_ap.ap,
        indirect_dim_max_index=in_ap.shape[0],
        offset_expr=[
            mybir.DynamicAccessPatternOffsetExpr(
                coef=coef,
                aff_expr=mybir.DynamicAccessPatternOffsetExprAffExpr(
                    kind="IndirectArgId", arg_id=1
                ),
            )
        ],
    )
    in_l.append(off_l[0])
    return gp.add_instruction(
        mybir.InstDMACopy(
            name=gp.bass.get_next_instruction_name(),
            queue="qPoolDynamic",
            mode="Copy",
            ins=in_l,
            outs=out_l,
            oob_is_err=True,
            cce_op=mybir.AluOpType.bypass,
        )
    )


def _build_nc():
    nc = bacc.Bacc("TRN2", target_bir_lowering=False, debug=False, num_devices=NCORES)

    table = nc.dram_tensor("table", [E + SENT_PAD, 2], I32, kind="ExternalInput")
    r0 = nc.dram_tensor("r0", [P, C], I32, kind="ExternalInput")
    d0 = nc.dram_tensor("d0", [P, C], I32, kind="ExternalInput")
    u8 = nc.dram_tensor("u8", [L_TOTAL, P, C], F32, kind="ExternalInput")
    walks = nc.dram_tensor("walks", [L_TOTAL - 1, P, C], I32, kind="ExternalOutput")

    op = mybir.AluOpType
    with tile.TileContext(nc) as tc:
        with tc.tile_pool(name="sbuf", bufs=1) as pool:
            u_sb = pool.tile([P, L_TOTAL, C], F32)
            r_cur = pool.tile([P, C], I32, tag="rcur")
            d_cur = pool.tile([P, C], I32, tag="dcur")
            walks_sb = pool.tile([P, L_TOTAL - 1, C], I32)

            nc.sync.dma_start(u_sb[:], u8.ap().rearrange("t p c -> p t c"))
            nc.sync.dma_start(r_cur[:], r0.ap())
            nc.sync.dma_start(d_cur[:], d0.ap())

            for t in range(L_TOTAL):
                df = pool.tile([P, C], F32, tag="df")
                prod = pool.tile([P, C], F32, tag="prod")
                oi = pool.tile([P, C], I32, tag="oi")
                of = pool.tile([P, C], F32, tag="of")
                gt = pool.tile([P, C], I32, tag="gt")
                off = pool.tile([P, C], I32, tag="off")
                dm = pool.tile([P, C], I32, tag="dm")
                e = pool.tile([P, C], I32, tag="e")
                g = pool.tile([P, C, 2], I32, tag="g", bufs=2)

                nc.vector.tensor_copy(df[:], d_cur[:])
                nc.vector.tensor_tensor(prod[:], u_sb[:, t, :], df[:], op=op.mult)
                nc.vector.tensor_copy(oi[:], prod[:])
                nc.vector.tensor_copy(of[:], oi[:])
                nc.vector.tensor_tensor(gt[:], of[:], prod[:], op=op.is_gt)
                nc.vector.tensor_tensor(off[:], oi[:], gt[:], op=op.subtract)
                nc.vector.tensor_scalar(dm[:], d_cur[:], -1, 0, op.add, op.max)
                nc.vector.tensor_tensor(off[:], off[:], dm[:], op=op.min)
                nc.vector.tensor_tensor(e[:], r_cur[:], off[:], op=op.add)

                for i in range(C):
                    _indirect_gather_flat(
                        nc.gpsimd, g[:, i, :], table.ap(), e[:, i : i + 1]
                    )
                    if (i + 1) % BARRIER_EVERY == 0:
                        tc.strict_bb_all_engine_barrier()
                tc.strict_bb_all_engine_barrier()

                w0 = g[:, :, 0:1]
                w1 = g[:, :, 1:2]
                if t >= 1:
                    nc.vector.tensor_scalar(
                        walks_sb[:, t - 1, :], w0, (1 << DEG_SHIFT) - 1, None,
                        op.bitwise_and,
                    )
                if t < L_TOTAL - 1:
                    nc.vector.tensor_scalar(
                        d_cur[:], w0, DEG_SHIFT, None, op.logical_shift_right
                    )
                    nc.vector.tensor_copy(r_cur[:], w1)

            nc.sync.dma_start(walks.ap().rearrange("t p c -> p t c"), walks_sb[:])

    nc.compile()
    return nc


def kernel(edge_index: np.ndarray, edge_weight: np.ndarray, rand_unif: np.ndarray):
    global LAST_EXEC_TIME_NS, LAST_RESULTS
    edge_index = np.asarray(edge_index, dtype=np.int32)
    edge_weight = np.asarray(edge_weight, dtype=np.float32)
    rand_unif = np.asarray(rand_unif, dtype=np.float32)

    row, col = edge_index[0], edge_index[1]
    order = np.argsort(row, kind="stable")
    col_s = col[order].astype(np.int32)
    deg = np.bincount(row, minlength=N).astype(np.int32)
    rowptr = np.zeros(N, dtype=np.int32)
    np.cumsum(deg[:-1], out=rowptr[1:])
    assert int(deg.max()) < 128, "deg must fit in 7 bits for word0 packing"

    # Isolated nodes walk in place via sentinel table rows.
    iso = np.flatnonzero(deg == 0).astype(np.int32)
    assert len(iso) <= SENT_PAD, f"too many isolated nodes: {len(iso)}"
    rowptr2 = rowptr.copy()
    rowptr2[iso] = E + np.arange(len(iso), dtype=np.int32)

    table = np.zeros((E + SENT_PAD, 2), dtype=np.int32)
    table[:E, 0] = col_s + (deg[col_s] << DEG_SHIFT)
    table[:E, 1] = rowptr2[col_s]
    if len(iso):
        table[E : E + len(iso), 0] = iso  # deg 0
        table[E : E + len(iso), 1] = E + np.arange(len(iso), dtype=np.int32)

    start_all = np.tile(np.arange(N, dtype=np.int32), WALKS_PER_NODE)

    in_maps = []
    for c in range(NCORES):
        starts = start_all[c * SPC : (c + 1) * SPC]
        r0 = np.zeros(SPAD, dtype=np.int32)
        d0 = np.ones(SPAD, dtype=np.int32)  # pad walkers: deg=1, rowptr=0
        r0[:SPC] = rowptr2[starts]
        d0[:SPC] = deg[starts]
        u = np.zeros((L_TOTAL, SPAD), dtype=np.float32)
        u[:, :SPC] = rand_unif[c * SPC : (c + 1) * SPC].T
        in_maps.append(
            {
                "table": table,
                "r0": r0.reshape(P, C),
                "d0": d0.reshape(P, C),
                "u8": np.ascontiguousarray(u.reshape(L_TOTAL, P, C)),
            }
        )

    nc = _build_nc()
    trace = bool(int(os.environ.get("KERNEL_TRACE", "0")))
    if trace:
        try:
            import profhook

            profhook.install()
        except Exception:
            trace = False
    res = run_bass_kernel_spmd(nc, in_maps, core_ids=list(range(NCORES)), trace=trace)
    LAST_EXEC_TIME_NS = res.exec_time_ns
    LAST_RESULTS = res

    targets = np.empty((S, L_TOTAL - 1), dtype=np.int32)
    for c in range(NCORES):
        w = res.results[c]["walks"].reshape(L_TOTAL - 1, SPAD)
        targets[c * SPC : (c + 1) * SPC] = w[:, :SPC].T

    roots = np.repeat(start_all[:, None], L_TOTAL - 1, axis=1)
    added = np.stack([roots.reshape(-1), targets.reshape(-1)])
    out_edge_index = np.concatenate([edge_index, added], axis=1)
    out_edge_weight = np.concatenate(
        [edge_weight, np.ones(added.shape[1], dtype=np.float32)]
    )
    return out_edge_index, out_edge_weight


# revision 9
# speedup vs baseline: 1.3319x; 1.3319x over previous
"""AddRandomWalkEdge on 8 TRN2 NeuronCores.

Strategy (per the sharding hint): shard the 400k walkers across the 8 cores,
replicate the CSR-derived table on each core, concatenate per-core walk
outputs on the host.

Host side (graph preprocessing only — no sampling):
  - stable-sort edges by source to build CSR (matches jnp.argsort(stable)),
  - build a fused per-edge table T[e] = (col_s[e] | deg[col_s[e]]<<17,
    rowptr[col_s[e]]) packed in 8 bytes, so each walk step needs exactly ONE
    8-byte gather per walker,
  - isolated nodes get sentinel table rows so the device loop is branch-free.

Device side (the sampling): for each of 8 steps, VectorE computes the edge
index e = rowptr[cur] + min(floor(u * deg), max(deg-1, 0)) for each walker,
then indirect DMAs gather T[e] from HBM. The floor is computed with a
rounding-mode-agnostic correction (cast, compare, subtract) so it matches
numpy/XLA truncation exactly.

The walrus dynamic-DMA contract (established by probing this compiler):
one instruction consumes ONE offset per dest partition (the first element
of that partition's offset-AP row) and copies a contiguous elems-sized run
from table[offset] into that partition's dest run. So a 50176-walker gather
needs 392 instructions of 128 walkers each (dest g[:, 2i:2i+2], offsets
e[:, i:i+1]). More than ~320 dynamic DMAs in flight crash the device, so an
all-engine barrier is inserted every 128 gathers.
"""

import os

import numpy as np

import concourse.bacc as bacc
import concourse.bass as bass
import concourse.mybir as mybir
import concourse.tile as tile
from concourse.bass_utils import run_bass_kernel_spmd

N = 100000
E = 3_200_000
WALKS_PER_NODE = 4
L_TOTAL = 8
S = N * WALKS_PER_NODE
NCORES = 8
SPC = S // NCORES  # walkers per core
P = 128
C = 391  # walker columns per plane; 128*391 = 50048 >= 50000
SPAD = P * C
SENT_PAD = 128  # sentinel rows for isolated nodes
DEG_SHIFT = 17  # word0 = col | deg << 17 (col < 2^17, deg < 2^7)
BARRIER_EVERY = 256  # flow control: >~320 dynamic DMAs in flight crash the device
CB = 4  # column blocks per step for unpack/compute pipelining

I32 = mybir.dt.int32
F32 = mybir.dt.float32

LAST_EXEC_TIME_NS = None
LAST_RESULTS = None


def _indirect_gather_flat(gp, out_ap, in_ap, offset_ap):
    """Dynamic DMA: per dest partition p, copy `elems` contiguous elements
    from in_[off_p] (off_p = first element of partition p's offset-AP row)
    into that partition's dest run. Bass's stock indirect_dma_start with the
    dest-space assert relaxed."""
    out_l = gp.lower_ap_dma(out_ap, for_indirect_dma=True)
    in_l = gp.lower_ap_dma(in_ap, for_indirect_dma=True)
    assert len(in_l) == 1 and len(out_l) == 1
    off_l = gp.lower_ap_dma(offset_ap)
    assert len(off_l) == 1

    coef = 1
    for i in range(1, len(in_ap.shape)):
        coef *= in_ap.shape[i]

    in_l[0].dynamic_ap_info = mybir.DynamicAccessPatternInfo(
        c=0,
        actual_ap=out_ap.ap,
        indirect_dim_max_index=in_ap.shape[0],
        offset_expr=[
            mybir.DynamicAccessPatternOffsetExpr(
                coef=coef,
                aff_expr=mybir.DynamicAccessPatternOffsetExprAffExpr(
                    kind="IndirectArgId", arg_id=1
                ),
            )
        ],
    )
    in_l.append(off_l[0])
    return gp.add_instruction(
        mybir.InstDMACopy(
            name=gp.bass.get_next_instruction_name(),
            queue="qPoolDynamic",
            mode="Copy",
            ins=in_l,
            outs=out_l,
            oob_is_err=True,
            cce_op=mybir.AluOpType.bypass,
        )
    )


def _build_nc():
    nc = bacc.Bacc("TRN2", target_bir_lowering=False, debug=False, num_devices=NCORES)

    table = nc.dram_tensor("table", [E + SENT_PAD, 2], I32, kind="ExternalInput")
    r0 = nc.dram_tensor("r0", [P, C], I32, kind="ExternalInput")
    d0 = nc.dram_tensor("d0", [P, C], I32, kind="ExternalInput")
    u8 = nc.dram_tensor("u8", [L_TOTAL, P, C], F32, kind="ExternalInput")
    walks = nc.dram_tensor("walks", [L_TOTAL - 1, P, C], I32, kind="ExternalOutput")

    op = mybir.AluOpType
    BS = (C + CB - 1) // CB  # columns per block
    MASK = (1 << DEG_SHIFT) - 1
    with tile.TileContext(nc) as tc:
        with tc.tile_pool(name="sbuf", bufs=1) as pool:
            u_sb = pool.tile([P, L_TOTAL, C], F32)
            r_cur = pool.tile([P, C], I32, tag="rcur")
            d_cur = pool.tile([P, C], I32, tag="dcur")
            walks_sb = pool.tile([P, L_TOTAL - 1, C], I32)

            nc.sync.dma_start(u_sb[:], u8.ap().rearrange("t p c -> p t c"))
            nc.sync.dma_start(r_cur[:], r0.ap())
            nc.sync.dma_start(d_cur[:], d0.ap())

            g_prev = None
            for t in range(L_TOTAL):
                df = pool.tile([P, C], F32, tag="df", bufs=2)
                prod = pool.tile([P, C], F32, tag="prod", bufs=2)
                oi = pool.tile([P, C], I32, tag="oi", bufs=2)
                of = pool.tile([P, C], F32, tag="of", bufs=2)
                gt = pool.tile([P, C], I32, tag="gt", bufs=2)
                off = pool.tile([P, C], I32, tag="off", bufs=2)
                dm = pool.tile([P, C], I32, tag="dm", bufs=2)
                e = pool.tile([P, C], I32, tag="e", bufs=2)
                g = pool.tile([P, C, 2], I32, tag="g", bufs=2)

                # Per column-block: unpack the previous step's gather for this
                # block, then compute this step's edge indices for it. Blocks
                # only depend on their own columns' gathers, so this DVE work
                # overlaps the tail of the previous step's gather stream.
                for b in range(CB):
                    sl = slice(b * BS, min((b + 1) * BS, C))
                    if t >= 1:
                        w0 = g_prev[:, sl, 0:1]
                        w1 = g_prev[:, sl, 1:2]
                        if t >= 2:
                            nc.vector.tensor_scalar(
                                walks_sb[:, t - 2, sl], w0, MASK, None, op.bitwise_and
                            )
                        nc.vector.tensor_scalar(
                            d_cur[:, sl], w0, DEG_SHIFT, None, op.logical_shift_right
                        )
                        nc.vector.tensor_copy(r_cur[:, sl], w1)
                    nc.vector.tensor_copy(df[:, sl], d_cur[:, sl])
                    nc.vector.tensor_tensor(
                        prod[:, sl], u_sb[:, t, sl], df[:, sl], op=op.mult
                    )
                    nc.vector.tensor_copy(oi[:, sl], prod[:, sl])
                    nc.vector.tensor_copy(of[:, sl], oi[:, sl])
                    nc.vector.tensor_tensor(
                        gt[:, sl], of[:, sl], prod[:, sl], op=op.is_gt
                    )
                    nc.vector.tensor_tensor(
                        off[:, sl], oi[:, sl], gt[:, sl], op=op.subtract
                    )
                    nc.vector.tensor_scalar(
                        dm[:, sl], d_cur[:, sl], -1, 0, op.add, op.max
                    )
                    nc.vector.tensor_tensor(
                        off[:, sl], off[:, sl], dm[:, sl], op=op.min
                    )
                    nc.vector.tensor_tensor(
                        e[:, sl], r_cur[:, sl], off[:, sl], op=op.add
                    )

                for i in range(C):
                    _indirect_gather_flat(
                        nc.gpsimd, g[:, i, :], table.ap(), e[:, i : i + 1]
                    )
                    if (i + 1) % BARRIER_EVERY == 0:
                        tc.strict_bb_all_engine_barrier()
                g_prev = g

            for b in range(CB):
                sl = slice(b * BS, min((b + 1) * BS, C))
                nc.vector.tensor_scalar(
                    walks_sb[:, L_TOTAL - 2, sl], g_prev[:, sl, 0:1], MASK, None,
                    op.bitwise_and,
                )

            nc.sync.dma_start(walks.ap().rearrange("t p c -> p t c"), walks_sb[:])

    nc.compile()
    return nc


def kernel(edge_index: np.ndarray, edge_weight: np.ndarray, rand_unif: np.ndarray):
    global LAST_EXEC_TIME_NS, LAST_RESULTS
    edge_index = np.asarray(edge_index, dtype=np.int32)
    edge_weight = np.asarray(edge_weight, dtype=np.float32)
    rand_unif = np.asarray(rand_unif, dtype=np.float32)

    row, col = edge_index[0], edge_index[1]
    order = np.argsort(row, kind="stable")
    col_s = col[order].astype(np.int32)
    deg = np.bincount(row, minlength=N).astype(np.int32)
    rowptr = np.zeros(N, dtype=np.int32)
    np.cumsum(deg[:-1], out=rowptr[1:])
    assert int(deg.max()) < 128, "deg must fit in 7 bits for word0 packing"

    # Isolated nodes walk in place via sentinel table rows.
    iso = np.flatnonzero(deg == 0).astype(np.int32)
    assert len(iso) <= SENT_PAD, f"too many isolated nodes: {len(iso)}"
    rowptr2 = rowptr.copy()
    rowptr2[iso] = E + np.arange(len(iso), dtype=np.int32)

    table = np.zeros((E + SENT_PAD, 2), dtype=np.int32)
    table[:E, 0] = col_s + (deg[col_s] << DEG_SHIFT)
    table[:E, 1] = rowptr2[col_s]
    if len(iso):
        table[E : E + len(iso), 0] = iso  # deg 0
        table[E : E + len(iso), 1] = E + np.arange(len(iso), dtype=np.int32)

    start_all = np.tile(np.arange(N, dtype=np.int32), WALKS_PER_NODE)

    in_maps = []
    for c in range(NCORES):
        starts = start_all[c * SPC : (c + 1) * SPC]
        r0 = np.zeros(SPAD, dtype=np.int32)
        d0 = np.ones(SPAD, dtype=np.int32)  # pad walkers: deg=1, rowptr=0
        r0[:SPC] = rowptr2[starts]
        d0[:SPC] = deg[starts]
        u = np.zeros((L_TOTAL, SPAD), dtype=np.float32)
        u[:, :SPC] = rand_unif[c * SPC : (c + 1) * SPC].T
        in_maps.append(
            {
                "table": table,
                "r0": r0.reshape(P, C),
                "d0": d0.reshape(P, C),
                "u8": np.ascontiguousarray(u.reshape(L_TOTAL, P, C)),
            }
        )

    nc = _build_nc()
    trace = bool(int(os.environ.get("KERNEL_TRACE", "0")))
    if trace:
        try:
            import profhook

            profhook.install()
        except Exception:
            trace = False
    res = run_bass_kernel_spmd(nc, in_maps, core_ids=list(range(NCORES)), trace=trace)
    LAST_EXEC_TIME_NS = res.exec_time_ns
    LAST_RESULTS = res

    targets = np.empty((S, L_TOTAL - 1), dtype=np.int32)
    for c in range(NCORES):
        w = res.results[c]["walks"].reshape(L_TOTAL - 1, SPAD)
        targets[c * SPC : (c + 1) * SPC] = w[:, :SPC].T

    roots = np.repeat(start_all[:, None], L_TOTAL - 1, axis=1)
    added = np.stack([roots.reshape(-1), targets.reshape(-1)])
    out_edge_index = np.concatenate([edge_index, added], axis=1)
    out_edge_weight = np.concatenate(
        [edge_weight, np.ones(added.shape[1], dtype=np.float32)]
    )
    return out_edge_index, out_edge_weight
